# revision 4
# baseline (speedup 1.0000x reference)
import sys
import math

if "/opt/trn_rl_repo" not in sys.path:
    sys.path.insert(0, "/opt/trn_rl_repo")

import numpy as np
from contextlib import ExitStack

import concourse.bass as bass
import concourse.bacc as bacc
import concourse.mybir as mybir
import concourse.tile as tile
from concourse.bass_utils import run_bass_kernel_spmd

F32 = mybir.dt.float32
F16 = mybir.dt.float16
BF16 = mybir.dt.bfloat16
EXP = mybir.ActivationFunctionType.Exp
MULT = mybir.AluOpType.mult
ADD = mybir.AluOpType.add
AXX = mybir.AxisListType.X

B, H, L, D, M = 8, 4, 4096, 128, 640
NCORES = 8
NBH = (B * H) // NCORES
NEG_GSCALE = -1.0 / (2.0 * math.sqrt(D))


def build_bass(n_bh=NBH, seq=L):
    nc = bacc.Bacc("TRN2", debug=False)
    q = nc.dram_tensor("q", [n_bh, seq, D], F16, kind="ExternalInput").ap()
    k = nc.dram_tensor("k", [n_bh, seq, D], F16, kind="ExternalInput").ap()
    v = nc.dram_tensor("v", [n_bh, seq, D], F16, kind="ExternalInput").ap()
    projT = nc.dram_tensor("projT", [D, M], F16, kind="ExternalInput").ap()
    out = nc.dram_tensor("out", [n_bh, seq, D], F32, kind="ExternalOutput").ap()

    assert seq % 1024 == 0
    nsg = seq // 1024
    ntile = 8 * nsg

    def ldma(sbuf_tile, dram_ap, sg):
        nc.sync.dma_start(
            sbuf_tile[:],
            dram_ap[1024 * sg : 1024 * (sg + 1), :].rearrange(
                "(t p) d -> p t d", t=8, p=128
            ),
        )

    def ldma_t(sbuf_tile, dram_ap, sg):
        nc.sync.dma_start_transpose(
            sbuf_tile[:], dram_ap[1024 * sg : 1024 * (sg + 1), :]
        )

    with tile.TileContext(nc) as tc, ExitStack() as ctx:
        const = ctx.enter_context(tc.tile_pool(name="const", bufs=1))
        warm = const.tile([1, 2], F32)
        nc.vector.memset(warm[:, 0:1], 0.0)
        nc.scalar.activation(warm[:, 1:2], warm[:, 0:1], EXP, bias=0.0, scale=1.0)
        projT_sb = const.tile([D, M], F16)
        nc.sync.dma_start(projT_sb[:], projT)

        ld_k = ctx.enter_context(tc.tile_pool(name="ld_k", bufs=2))
        ld_v = ctx.enter_context(tc.tile_pool(name="ld_v", bufs=2))
        kt_p = ctx.enter_context(tc.tile_pool(name="kt_sb", bufs=2))
        qt_p = ctx.enter_context(tc.tile_pool(name="qt_sb", bufs=2))
        phik_p = ctx.enter_context(tc.tile_pool(name="phik", bufs=6))
        phiq_p = ctx.enter_context(tc.tile_pool(name="phiq", bufs=8))
        misc_p = ctx.enter_context(tc.tile_pool(name="misc", bufs=2))
        acc_p = ctx.enter_context(tc.tile_pool(name="acc", bufs=2))
        ctxsb_p = ctx.enter_context(tc.tile_pool(name="ctxsb", bufs=2))
        epi_p = ctx.enter_context(tc.tile_pool(name="episb", bufs=1))
        outsb_p = ctx.enter_context(tc.tile_pool(name="outsb", bufs=2))

        def g_rowsums(k_view, negb, nt, tag_sfx=""):
            gscr = misc_p.tile(
                [128, nt, D], F16, tag=f"gscr{tag_sfx}", name=f"gscr{tag_sfx}_{nc.next_id()}"
            )
            nc.vector.tensor_mul(gscr[:], k_view, k_view)
            with nc.allow_low_precision(reason="f32 accumulate of fp16 squares"):
                nc.vector.reduce_sum(negb[:, 0:nt], gscr[:], axis=AXX)
            nc.vector.tensor_scalar_mul(negb[:, 0:nt], negb[:, 0:nt], NEG_GSCALE)

        def k_prologue(bh, cold=False):
            v_buf = ld_v.tile([128, 8, D], F16, tag="v", name=f"v_buf0_{bh}")
            if cold:
                kt_a = kt_p.tile([128, 512], F16, tag="kt_a", bufs=1)
                kt_b = kt_p.tile([128, 512], F16, tag="kt_b", bufs=1)
                k_a = ld_k.tile([128, 4, D], F16, tag="k_a", bufs=1)
                k_b = ld_k.tile([128, 4, D], F16, tag="k_b", bufs=1)
                negb_a = misc_p.tile([128, 4], F32, tag="negb_a", bufs=1)
                negb_b = misc_p.tile([128, 4], F32, tag="negb_b", bufs=1)
                nc.sync.dma_start(
                    k_a[:], k[bh][0:512, :].rearrange("(t p) d -> p t d", t=4, p=128)
                )
                nc.sync.dma_start_transpose(kt_a[:], k[bh][0:512, :])
                nc.sync.dma_start(
                    k_b[:], k[bh][512:1024, :].rearrange("(t p) d -> p t d", t=4, p=128)
                )
                nc.sync.dma_start_transpose(kt_b[:], k[bh][512:1024, :])
                ldma(v_buf, v[bh], 0)
                g_rowsums(k_a[:], negb_a, 4, "_a")
                g_rowsums(k_b[:], negb_b, 4, "_b")
                return (
                    [(kt_a, 0, 4), (kt_b, 4, 4)],
                    [(negb_a, 0), (negb_b, 4)],
                    v_buf,
                )
            k_buf = ld_k.tile([128, 8, D], F16, tag="k", name=f"k_buf0_{bh}")
            kt_sb = kt_p.tile([128, 1024], F16, tag="kt", name=f"kt_sb0_{bh}")
            ldma_t(kt_sb, k[bh], 0)
            ldma(k_buf, k[bh], 0)
            ldma(v_buf, v[bh], 0)
            negb = misc_p.tile([128, 8], F32, tag="negb", name=f"negb0_{bh}")
            g_rowsums(k_buf[:], negb, 8)
            return [(kt_sb, 0, 8)], [(negb, 0)], v_buf

        def pre_lookup(parts, u):
            for tile_, base, nt in parts:
                if base <= u < base + nt:
                    return tile_, u - base
            raise AssertionError

        preloaded = k_prologue(0, cold=True)

        for bh in range(n_bh):
            acc_a = acc_p.tile([128, M], F16, tag="acc_a")
            acc_b = acc_p.tile([128, M], F16, tag="acc_b")
            ctxT_sb = ctxsb_p.tile([128, M], F16, tag="ctxT")
            qt_sb0 = None
            with tc.tile_pool(name="ps_ctx", bufs=1, space="PSUM") as ps_ctx, \
                 tc.tile_pool(name="ps_arr", bufs=2, space="PSUM") as ps_arr:
                ctxT_ps = ps_ctx.tile([128, M], F32)
                for sg in range(nsg):
                    if sg == nsg - 1:
                        qt_sb0 = qt_p.tile([128, 1024], F16, tag="qt")
                        ldma_t(qt_sb0, q[bh], 0)
                    if sg == 0:
                        kt_parts, negb_parts, v_buf = preloaded
                    else:
                        k_buf = ld_k.tile([128, 8, D], F16, tag="k")
                        kt_sb = kt_p.tile([128, 1024], F16, tag="kt")
                        ldma_t(kt_sb, k[bh], sg)
                        ldma(k_buf, k[bh], sg)
                        v_buf = ld_v.tile([128, 8, D], F16, tag="v")
                        ldma(v_buf, v[bh], sg)
                        negb = misc_p.tile([128, 8], F32, tag="negb")
                        g_rowsums(k_buf[:], negb, 8)
                        kt_parts = [(kt_sb, 0, 8)]
                        negb_parts = [(negb, 0)]
                    for u in range(8):
                        gi = 8 * sg + u
                        arr = ps_arr.tile([128, M], F32, tag="arr")
                        kt_t, ku = pre_lookup(kt_parts, u)
                        lhsT = kt_t[:, 128 * ku : 128 * (ku + 1)]
                        nc.tensor.matmul(arr[:, 0:512], lhsT, projT_sb[:, 0:512])
                        nc.tensor.matmul(arr[:, 512:M], lhsT, projT_sb[:, 512:M])
                        phik = phik_p.tile([128, M], F16, tag="phik")
                        negb_t, nu = pre_lookup(
                            [(t, b, 8) for t, b in negb_parts]
                            if len(negb_parts) == 1
                            else [(t, b, 4) for t, b in negb_parts],
                            u,
                        )
                        nc.scalar.activation(
                            phik[:], arr[:], EXP, bias=negb_t[:, nu : nu + 1], scale=1.0
                        )
                        first = gi == 0
                        last = gi == ntile - 1
                        nc.tensor.matmul(
                            ctxT_ps[:, 0:512], v_buf[:, u, :], phik[:, 0:512],
                            start=first, stop=last,
                        )
                        nc.tensor.matmul(
                            ctxT_ps[:, 512:M], v_buf[:, u, :], phik[:, 512:M],
                            start=first, stop=last,
                        )
                        if gi == 0:
                            nc.vector.tensor_copy(acc_a[:], phik[:])
                        elif gi == 1:
                            nc.gpsimd.tensor_copy(acc_b[:], phik[:])
                        elif gi % 2 == 0:
                            nc.vector.tensor_add(acc_a[:], acc_a[:], phik[:])
                        else:
                            nc.gpsimd.tensor_add(acc_b[:], acc_b[:], phik[:])
                nc.vector.tensor_copy(ctxT_sb[:], ctxT_ps[:])
                nc.vector.tensor_add(acc_a[:], acc_a[:], acc_b[:])

            ctx_aug = ctxsb_p.tile([128, 5, 129], BF16, tag="ctx_aug")
            with tc.tile_pool(name="ps_nd", bufs=2, space="PSUM") as ps_nd, \
                 tc.tile_pool(name="ps_arrq", bufs=2, space="PSUM") as ps_arrq:

                def q_head(qt_sb, p):
                    arrq = ps_arrq.tile([128, 2, 5, 128], F32, tag="arrq")
                    for i in range(2):
                        rhs = qt_sb[:, 256 * p + 128 * i : 256 * p + 128 * (i + 1)]
                        for j in range(5):
                            nc.tensor.matmul(
                                arrq[:, i, j, :],
                                projT_sb[:, 128 * j : 128 * (j + 1)],
                                rhs,
                            )
                    phiq = phiq_p.tile([128, 2, 5, 128], BF16, tag="phiq")
                    nc.scalar.activation(phiq[:], arrq[:], EXP, bias=0.0, scale=1.0)
                    return phiq

                def q_tail(phiq, out_sb, p):
                    for i in range(2):
                        nd = ps_nd.tile([128, 129], F32, tag="nd")
                        for j in range(5):
                            nc.tensor.matmul(
                                nd[:],
                                phiq[:, i, j, :],
                                ctx_aug[:, j, :],
                                start=(j == 0), stop=(j == 4),
                            )
                        recip = misc_p.tile([128, 1], F32, tag="recip")
                        nc.vector.reciprocal(recip[:], nd[:, 128:129])
                        nc.vector.tensor_scalar_mul(
                            out_sb[:, 2 * p + i, :], nd[:, 0:128], recip[:]
                        )

                qt_sbs = {0: qt_sb0}
                out_sbs = {}
                pending = []

                def emit_head(b):
                    sg, p = divmod(b, 4)
                    if p == 0:
                        if sg not in qt_sbs:
                            qt_sbs[sg] = qt_p.tile(
                                [128, 1024], F16, tag="qt", name=f"qt_sb{sg}"
                            )
                            ldma_t(qt_sbs[sg], q[bh], sg)
                        out_sbs[sg] = outsb_p.tile(
                            [128, 8, D], F32, tag="out", name=f"out_sb{sg}"
                        )
                    pending.append((q_head(qt_sbs[sg], p), sg, p))

                def emit_tail():
                    phiq, sg, p = pending.pop(0)
                    q_tail(phiq, out_sbs[sg], p)
                    if p == 3:
                        nc.sync.dma_start(
                            out[bh, 1024 * sg : 1024 * (sg + 1), :].rearrange(
                                "(t p) d -> p t d", t=8, p=128
                            ),
                            out_sbs[sg][:],
                        )

                emit_head(0)
                emit_head(1)

                ksT = epi_p.tile([128, M], F16, tag="ksT")
                for j in range(5):
                    nc.sync.dma_start_transpose(
                        ksT[:, 128 * j : 128 * (j + 1)],
                        acc_a[:, 128 * j : 128 * (j + 1)],
                    )
                with nc.allow_low_precision(reason="bf16 ksum: el err averages out over m"):
                    for j in range(5):
                        nc.vector.reduce_sum(
                            ctx_aug[:, j, 128:129],
                            ksT[:, 128 * j : 128 * (j + 1)],
                            axis=AXX,
                        )
                fixT = epi_p.tile([128, M], F16, tag="fixT")
                for j in range(5):
                    nc.sync.dma_start_transpose(
                        fixT[:, 128 * j : 128 * (j + 1)],
                        ctxT_sb[:, 128 * j : 128 * (j + 1)],
                    )
                for j in range(5):
                    nc.vector.tensor_copy(
                        ctx_aug[:, j, 0:128], fixT[:, 128 * j : 128 * (j + 1)]
                    )

                for b in range(2, 4 * nsg):
                    emit_tail()
                    emit_head(b)
                    if b == 4 * nsg - 3 and bh + 1 < n_bh:
                        preloaded = k_prologue(bh + 1)
                emit_tail()
                emit_tail()
    nc.compile()
    return nc


_NC_CACHE = {}


def _get_nc(n_bh=NBH, seq=L):
    key = (n_bh, seq)
    if key not in _NC_CACHE:
        _NC_CACHE[key] = build_bass(n_bh, seq)
    return _NC_CACHE[key]


def host_inputs(projection_matrix):
    projT = np.ascontiguousarray(
        (np.asarray(projection_matrix, dtype=np.float32) / (D**0.25)).T
    ).astype(np.float16)
    return projT


def kernel(q, k, v, projection_matrix, _trace=False, _trace_kwargs=None):
    q = np.asarray(q, dtype=np.float32).reshape(B * H, L, D).astype(np.float16)
    k = np.asarray(k, dtype=np.float32).reshape(B * H, L, D).astype(np.float16)
    v = np.asarray(v, dtype=np.float32).reshape(B * H, L, D).astype(np.float16)
    projT = host_inputs(projection_matrix)

    in_maps = []
    for c in range(NCORES):
        sl = slice(NBH * c, NBH * (c + 1))
        in_maps.append(
            {
                "q": np.ascontiguousarray(q[sl]),
                "k": np.ascontiguousarray(k[sl]),
                "v": np.ascontiguousarray(v[sl]),
                "projT": projT,
            }
        )

    nc = _get_nc()
    kwargs = {}
    if _trace:
        kwargs["trace"] = True
        kwargs.update(_trace_kwargs or {})
    res = run_bass_kernel_spmd(nc, in_maps, core_ids=list(range(NCORES)), **kwargs)
    outs = np.concatenate([res.results[c]["out"] for c in range(NCORES)], axis=0)
    result = outs.reshape(B, H, L, D).astype(np.float32)
    if _trace:
        return result, res
    return result


def timed_run(q, k, v, projection_matrix, iters=5):
    import time
    import jax
    from jax.sharding import Mesh, PartitionSpec
    from jax.experimental.shard_map import shard_map
    from concourse import bass2jax

    q = np.asarray(q, dtype=np.float32).reshape(B * H, L, D).astype(np.float16)
    k = np.asarray(k, dtype=np.float32).reshape(B * H, L, D).astype(np.float16)
    v = np.asarray(v, dtype=np.float32).reshape(B * H, L, D).astype(np.float16)
    projT = host_inputs(projection_matrix)
    nc = _get_nc()
    bass2jax.install_neuronx_cc_hook()

    in_names = []
    out_names = []
    out_avals = []
    zero_outs = []
    import concourse.mybir as mybir_

    partition_name = nc.partition_id_tensor.name if nc.partition_id_tensor else None
    for alloc in nc.m.functions[0].allocations:
        if not isinstance(alloc, mybir_.MemoryLocationSet):
            continue
        name = alloc.memorylocations[0].name
        if alloc.kind == "ExternalInput":
            if name != partition_name:
                in_names.append(name)
        elif alloc.kind == "ExternalOutput":
            out_names.append(name)
            shape = list(alloc.tensor_shape)
            out_avals.append(jax.core.ShapedArray(shape, np.float32))
            zero_outs.append(np.zeros(shape, np.float32))
    n_params = len(in_names)
    n_outs = len(out_names)
    all_names = in_names + out_names
    if partition_name is not None:
        all_names = all_names + [partition_name]

    def _body(*args):
        operands = list(args)
        if partition_name is not None:
            operands.append(bass2jax.partition_id_tensor())
        outs = bass2jax._bass_exec_p.bind(
            *operands,
            out_avals=tuple(out_avals),
            in_names=tuple(all_names),
            out_names=tuple(out_names),
            lowering_input_output_aliases=(),
            sim_require_finite=True,
            sim_require_nnan=True,
            nc=nc,
        )
        return tuple(outs)

    devices = jax.devices()[:NCORES]
    mesh = Mesh(np.asarray(devices), ("core",))
    in_specs = (PartitionSpec("core"),) * (n_params + n_outs)
    out_specs = (PartitionSpec("core"),) * n_outs
    sharded = jax.jit(
        shard_map(_body, mesh=mesh, in_specs=in_specs, out_specs=out_specs, check_rep=False),
        keep_unused=True,
    )

    per_core_vals = {
        "q": [q[NBH * c : NBH * (c + 1)] for c in range(NCORES)],
        "k": [k[NBH * c : NBH * (c + 1)] for c in range(NCORES)],
        "v": [v[NBH * c : NBH * (c + 1)] for c in range(NCORES)],
        "projT": [projT] * NCORES,
    }
    concat_in = [
        np.concatenate(per_core_vals[nm], axis=0) for nm in in_names
    ]
    concat_zeros = [
        np.zeros((NCORES * z.shape[0], *z.shape[1:]), z.dtype) for z in zero_outs
    ]
    sharding = jax.sharding.NamedSharding(mesh, PartitionSpec("core"))
    dev_in = [jax.device_put(a, sharding) for a in concat_in]
    dev_zero = [jax.device_put(a, sharding) for a in concat_zeros]
    r0 = sharded(*dev_in, *dev_zero)
    jax.block_until_ready(r0)
    times = []
    for _ in range(iters):
        t0 = time.perf_counter()
        rr = sharded(*dev_in, *dev_zero)
        jax.block_until_ready(rr)
        times.append(time.perf_counter() - t0)
    out = np.asarray(rr[out_names.index("out")]).reshape(NCORES, NBH, L, D)
    result = out.reshape(B, H, L, D)
    return result, times


# revision 5
# speedup vs baseline: 1.1528x; 1.1528x over previous
import sys
import math

if "/opt/trn_rl_repo" not in sys.path:
    sys.path.insert(0, "/opt/trn_rl_repo")

import numpy as np
from contextlib import ExitStack

import concourse.bass as bass
import concourse.bacc as bacc
import concourse.mybir as mybir
import concourse.tile as tile
from concourse.bass_utils import run_bass_kernel_spmd

F32 = mybir.dt.float32
F16 = mybir.dt.float16
BF16 = mybir.dt.bfloat16
EXP = mybir.ActivationFunctionType.Exp
MULT = mybir.AluOpType.mult
ADD = mybir.AluOpType.add
AXX = mybir.AxisListType.X

B, H, L, D, M = 8, 4, 4096, 128, 640
NCORES = 8
NBH = (B * H) // NCORES
NEG_GSCALE = -1.0 / (2.0 * math.sqrt(D))


def build_bass(n_bh=NBH, seq=L):
    nc = bacc.Bacc("TRN2", debug=False)
    q = nc.dram_tensor("q", [n_bh, seq, D], F16, kind="ExternalInput").ap()
    k = nc.dram_tensor("k", [n_bh, seq, D], F16, kind="ExternalInput").ap()
    v = nc.dram_tensor("v", [n_bh, seq, D], F16, kind="ExternalInput").ap()
    projT = nc.dram_tensor("projT", [D, M], F16, kind="ExternalInput").ap()
    out = nc.dram_tensor("out", [n_bh, seq, D], F32, kind="ExternalOutput").ap()

    assert seq % 1024 == 0
    nsg = seq // 1024
    ntile = 8 * nsg

    def ldma(sbuf_tile, dram_ap, sg):
        nc.sync.dma_start(
            sbuf_tile[:],
            dram_ap[1024 * sg : 1024 * (sg + 1), :].rearrange(
                "(t p) d -> p t d", t=8, p=128
            ),
        )

    def ldma_t(sbuf_tile, dram_ap, sg):
        nc.sync.dma_start_transpose(
            sbuf_tile[:], dram_ap[1024 * sg : 1024 * (sg + 1), :]
        )

    with tile.TileContext(nc) as tc, ExitStack() as ctx:
        const = ctx.enter_context(tc.tile_pool(name="const", bufs=1))
        warm = const.tile([1, 2], F32)
        nc.vector.memset(warm[:, 0:1], 0.0)
        nc.scalar.activation(warm[:, 1:2], warm[:, 0:1], EXP, bias=0.0, scale=1.0)
        projT_sb = const.tile([D, M], F16)
        nc.sync.dma_start(projT_sb[:], projT)

        ld_k = ctx.enter_context(tc.tile_pool(name="ld_k", bufs=2))
        ld_v = ctx.enter_context(tc.tile_pool(name="ld_v", bufs=2))
        kt_p = ctx.enter_context(tc.tile_pool(name="kt_sb", bufs=2))
        qt_p = ctx.enter_context(tc.tile_pool(name="qt_sb", bufs=2))
        phik_p = ctx.enter_context(tc.tile_pool(name="phik", bufs=6))
        phiq_p = ctx.enter_context(tc.tile_pool(name="phiq", bufs=8))
        misc_p = ctx.enter_context(tc.tile_pool(name="misc", bufs=2))
        acc_p = ctx.enter_context(tc.tile_pool(name="acc", bufs=2))
        ctxsb_p = ctx.enter_context(tc.tile_pool(name="ctxsb", bufs=2))
        epi_p = ctx.enter_context(tc.tile_pool(name="episb", bufs=1))
        outsb_p = ctx.enter_context(tc.tile_pool(name="outsb", bufs=2))
        vs_p = ctx.enter_context(tc.tile_pool(name="vs", bufs=2))

        def g_rowsums(k_view, negb, nt, tag_sfx=""):
            gscr = misc_p.tile(
                [128, nt, D], F16, tag=f"gscr{tag_sfx}", name=f"gscr{tag_sfx}_{nc.next_id()}"
            )
            for u in range(nt):
                nc.vector.scalar_tensor_tensor(
                    gscr[:, u, :], k_view[:, u, :], NEG_GSCALE, k_view[:, u, :],
                    op0=MULT, op1=MULT,
                    accum_out=negb[:, u : u + 1],
                )

        def sg_prep(bh, sg, cold=False):
            uid = f"{bh}_{sg}"
            v_buf = ld_v.tile([128, 8, D], F16, tag="v", name=f"v_buf_{uid}")
            if cold:
                kt_a = kt_p.tile([128, 512], F16, tag="kt_a", bufs=1)
                kt_b = kt_p.tile([128, 512], F16, tag="kt_b", bufs=1)
                k_a = ld_k.tile([128, 4, D], F16, tag="k_a", bufs=1)
                k_b = ld_k.tile([128, 4, D], F16, tag="k_b", bufs=1)
                negb_a = misc_p.tile([128, 4], F32, tag="negb_a", bufs=1)
                negb_b = misc_p.tile([128, 4], F32, tag="negb_b", bufs=1)
                nc.sync.dma_start(
                    k_a[:], k[bh][0:512, :].rearrange("(t p) d -> p t d", t=4, p=128)
                )
                nc.sync.dma_start_transpose(kt_a[:], k[bh][0:512, :])
                nc.sync.dma_start(
                    k_b[:], k[bh][512:1024, :].rearrange("(t p) d -> p t d", t=4, p=128)
                )
                nc.sync.dma_start_transpose(kt_b[:], k[bh][512:1024, :])
                ldma(v_buf, v[bh], 0)
                g_rowsums(k_a[:], negb_a, 4, "_a")
                g_rowsums(k_b[:], negb_b, 4, "_b")
                kt_parts = [(kt_a, 0, 4), (kt_b, 4, 4)]
                negb_parts = [(negb_a, 0), (negb_b, 4)]
            else:
                k_buf = ld_k.tile([128, 8, D], F16, tag="k", name=f"k_buf_{uid}")
                kt_sb = kt_p.tile([128, 1024], F16, tag="kt", name=f"kt_sb_{uid}")
                ldma_t(kt_sb, k[bh], sg)
                ldma(k_buf, k[bh], sg)
                ldma(v_buf, v[bh], sg)
                negb = misc_p.tile([128, 8], F32, tag="negb", name=f"negb_{uid}")
                g_rowsums(k_buf[:], negb, 8)
                kt_parts = [(kt_sb, 0, 8)]
                negb_parts = [(negb, 0)]
            return kt_parts, negb_parts, v_buf

        def pre_lookup(parts, u):
            for tile_, base, nt in parts:
                if base <= u < base + nt:
                    return tile_, u - base
            raise AssertionError

        preloaded = sg_prep(0, 0, cold=True)

        for bh in range(n_bh):
            acc_a = acc_p.tile([128, M], F16, tag="acc_a")
            acc_b = acc_p.tile([128, M], F16, tag="acc_b")
            ctxT_sb = ctxsb_p.tile([128, M], F16, tag="ctxT")
            qt_sb0 = None
            specials = []
            out_sb0 = None
            with tc.tile_pool(name="ps_ctx", bufs=1, space="PSUM") as ps_ctx, \
                 tc.tile_pool(name="ps_arr", bufs=2, space="PSUM") as ps_arr, \
                 tc.tile_pool(name="ps_q0", bufs=1, space="PSUM") as ps_q0:
                ctxT_ps = ps_ctx.tile([128, M], F32)
                next_prep = None
                for sg in range(nsg):
                    if sg == nsg - 1:
                        qt_sb0 = qt_p.tile([128, 1024], F16, tag="qt")
                        ldma_t(qt_sb0, q[bh], 0)
                    kt_parts, negb_parts, v_buf = preloaded if sg == 0 else next_prep
                    if sg + 1 < nsg:
                        next_prep = sg_prep(bh, sg + 1)
                    for u in range(8):
                        gi = 8 * sg + u
                        arr = ps_arr.tile([128, M], F32, tag="arr")
                        kt_t, ku = pre_lookup(kt_parts, u)
                        lhsT = kt_t[:, 128 * ku : 128 * (ku + 1)]
                        nc.tensor.matmul(arr[:, 0:512], lhsT, projT_sb[:, 0:512])
                        nc.tensor.matmul(arr[:, 512:M], lhsT, projT_sb[:, 512:M])
                        aq0s = []
                        if sg == nsg - 1 and u == 7:
                            out_sb0 = outsb_p.tile(
                                [128, 8, D], F32, tag="out", name="out_sb0q"
                            )
                            for t0 in range(2):
                                aq0 = ps_q0.tile([128, 5, 128], F32, tag="aq0")
                                for j in range(5):
                                    nc.tensor.matmul(
                                        aq0[:, j, :],
                                        projT_sb[:, 128 * j : 128 * (j + 1)],
                                        qt_sb0[:, 128 * t0 : 128 * (t0 + 1)],
                                    )
                                aq0s.append(aq0)
                        phik = phik_p.tile([128, M], F16, tag="phik")
                        negb_t, nu = pre_lookup(
                            [(t, b, 8 if len(negb_parts) == 1 else 4)
                             for t, b in negb_parts],
                            u,
                        )
                        nc.scalar.activation(
                            phik[:], arr[:], EXP, bias=negb_t[:, nu : nu + 1], scale=1.0
                        )
                        for t0, aq0 in enumerate(aq0s):
                            phiq0 = phiq_p.tile(
                                [128, 5, 128], BF16, tag="phiq0", bufs=2,
                                name=f"phiq0_{t0}",
                            )
                            nc.scalar.activation(
                                phiq0[:], aq0[:], EXP, bias=0.0, scale=1.0
                            )
                            specials.append(phiq0)
                        first = gi == 0
                        last = gi == ntile - 1
                        nc.tensor.matmul(
                            ctxT_ps[:, 0:512], v_buf[:, u, :], phik[:, 0:512],
                            start=first, stop=last,
                        )
                        nc.tensor.matmul(
                            ctxT_ps[:, 512:M], v_buf[:, u, :], phik[:, 512:M],
                            start=first, stop=last,
                        )
                        if gi == 0:
                            nc.vector.tensor_copy(acc_a[:], phik[:])
                        elif gi == 1:
                            nc.gpsimd.tensor_copy(acc_b[:], phik[:])
                        elif gi % 2 == 0:
                            nc.vector.tensor_add(acc_a[:], acc_a[:], phik[:])
                        else:
                            nc.gpsimd.tensor_add(acc_b[:], acc_b[:], phik[:])
                nc.vector.tensor_copy(ctxT_sb[:], ctxT_ps[:])
                nc.vector.tensor_add(acc_a[:], acc_a[:], acc_b[:])

            ctx_aug = ctxsb_p.tile([128, 5, 129], BF16, tag="ctx_aug")
            with tc.tile_pool(name="ps_nd", bufs=2, space="PSUM") as ps_nd, \
                 tc.tile_pool(name="ps_arrq", bufs=2, space="PSUM") as ps_arrq:

                def q_head(qt_sb, p):
                    arrq = ps_arrq.tile([128, 2, 5, 128], F32, tag="arrq")
                    for i in range(2):
                        rhs = qt_sb[:, 256 * p + 128 * i : 256 * p + 128 * (i + 1)]
                        for j in range(5):
                            nc.tensor.matmul(
                                arrq[:, i, j, :],
                                projT_sb[:, 128 * j : 128 * (j + 1)],
                                rhs,
                            )
                    phiq = phiq_p.tile([128, 2, 5, 128], BF16, tag="phiq")
                    nc.scalar.activation(phiq[:], arrq[:], EXP, bias=0.0, scale=1.0)
                    return phiq

                def nd_one(phiq_i, out_sb, u):
                    nd = ps_nd.tile([128, 129], F32, tag="nd")
                    for j in range(5):
                        nc.tensor.matmul(
                            nd[:],
                            phiq_i[:, j, :],
                            ctx_aug[:, j, :],
                            start=(j == 0), stop=(j == 4),
                        )
                    recip = misc_p.tile([128, 1], F32, tag="recip")
                    nc.vector.reciprocal(recip[:], nd[:, 128:129])
                    nc.vector.tensor_scalar_mul(
                        out_sb[:, u, :], nd[:, 0:128], recip[:]
                    )

                qt_sbs = {0: qt_sb0}
                out_sbs = {0: out_sb0}
                done_tiles = {s: 0 for s in range(nsg)}
                pending = [(specials[0], [0]), (specials[1], [1])]
                nbatch = 4 * nsg - 1

                def emit_head(b):
                    t0 = 2 + 2 * b
                    sg, p = t0 // 8, (t0 % 8) // 2
                    if sg not in qt_sbs:
                        qt_sbs[sg] = qt_p.tile(
                            [128, 1024], F16, tag="qt", name=f"qt_sb{sg}"
                        )
                        ldma_t(qt_sbs[sg], q[bh], sg)
                        out_sbs[sg] = outsb_p.tile(
                            [128, 8, D], F32, tag="out", name=f"out_sb{sg}"
                        )
                    pending.append((q_head(qt_sbs[sg], p), [t0, t0 + 1]))

                def store(sg, lo, hi):
                    nc.sync.dma_start(
                        out[bh, 1024 * sg + 128 * lo : 1024 * sg + 128 * hi, :]
                        .rearrange("(t p) d -> p t d", t=hi - lo, p=128),
                        out_sbs[sg][:, lo:hi, :],
                    )

                def emit_tail():
                    phiq, tiles = pending.pop(0)
                    for idx, t in enumerate(tiles):
                        sg = t // 8
                        pi = phiq[:, idx, :, :] if len(tiles) == 2 else phiq[:]
                        nd_one(pi, out_sbs[sg], t % 8)
                        done_tiles[sg] += 1
                        last_pair = bh == n_bh - 1 and sg == nsg - 1
                        if last_pair and done_tiles[sg] == 4:
                            store(sg, 0, 4)
                        elif done_tiles[sg] == 8:
                            store(sg, 4, 8) if last_pair else store(sg, 0, 8)

                emit_head(0)
                emit_head(1)

                ksT = epi_p.tile([128, M], F16, tag="ksT")
                for j in range(5):
                    nc.sync.dma_start_transpose(
                        ksT[:, 128 * j : 128 * (j + 1)],
                        acc_a[:, 128 * j : 128 * (j + 1)],
                    )
                with nc.allow_low_precision(reason="bf16 ksum: el err averages out over m"):
                    for j in range(5):
                        nc.vector.reduce_sum(
                            ctx_aug[:, j, 128:129],
                            ksT[:, 128 * j : 128 * (j + 1)],
                            axis=AXX,
                        )
                fixT = epi_p.tile([128, M], F16, tag="fixT")
                for j in range(5):
                    nc.sync.dma_start_transpose(
                        fixT[:, 128 * j : 128 * (j + 1)],
                        ctxT_sb[:, 128 * j : 128 * (j + 1)],
                    )
                for j in range(5):
                    nc.vector.tensor_copy(
                        ctx_aug[:, j, 0:128], fixT[:, 128 * j : 128 * (j + 1)]
                    )

                for b in range(2, nbatch):
                    emit_tail()
                    emit_head(b)
                    if b == nbatch - 2 and bh + 1 < n_bh:
                        preloaded = sg_prep(bh + 1, 0)
                while pending:
                    emit_tail()
    nc.compile()
    return nc


_NC_CACHE = {}


def _get_nc(n_bh=NBH, seq=L):
    key = (n_bh, seq)
    if key not in _NC_CACHE:
        _NC_CACHE[key] = build_bass(n_bh, seq)
    return _NC_CACHE[key]


def host_inputs(projection_matrix):
    projT = np.ascontiguousarray(
        (np.asarray(projection_matrix, dtype=np.float32) / (D**0.25)).T
    ).astype(np.float16)
    return projT


def kernel(q, k, v, projection_matrix, _trace=False, _trace_kwargs=None):
    q = np.asarray(q, dtype=np.float32).reshape(B * H, L, D).astype(np.float16)
    k = np.asarray(k, dtype=np.float32).reshape(B * H, L, D).astype(np.float16)
    v = np.asarray(v, dtype=np.float32).reshape(B * H, L, D).astype(np.float16)
    projT = host_inputs(projection_matrix)

    in_maps = []
    for c in range(NCORES):
        sl = slice(NBH * c, NBH * (c + 1))
        in_maps.append(
            {
                "q": np.ascontiguousarray(q[sl]),
                "k": np.ascontiguousarray(k[sl]),
                "v": np.ascontiguousarray(v[sl]),
                "projT": projT,
            }
        )

    nc = _get_nc()
    kwargs = {}
    if _trace:
        kwargs["trace"] = True
        kwargs.update(_trace_kwargs or {})
    res = run_bass_kernel_spmd(nc, in_maps, core_ids=list(range(NCORES)), **kwargs)
    outs = np.concatenate([res.results[c]["out"] for c in range(NCORES)], axis=0)
    result = outs.reshape(B, H, L, D).astype(np.float32)
    if _trace:
        return result, res
    return result


def timed_run(q, k, v, projection_matrix, iters=5):
    import time
    import jax
    from jax.sharding import Mesh, PartitionSpec
    from jax.experimental.shard_map import shard_map
    from concourse import bass2jax

    q = np.asarray(q, dtype=np.float32).reshape(B * H, L, D).astype(np.float16)
    k = np.asarray(k, dtype=np.float32).reshape(B * H, L, D).astype(np.float16)
    v = np.asarray(v, dtype=np.float32).reshape(B * H, L, D).astype(np.float16)
    projT = host_inputs(projection_matrix)
    nc = _get_nc()
    bass2jax.install_neuronx_cc_hook()

    in_names = []
    out_names = []
    out_avals = []
    zero_outs = []
    import concourse.mybir as mybir_

    partition_name = nc.partition_id_tensor.name if nc.partition_id_tensor else None
    for alloc in nc.m.functions[0].allocations:
        if not isinstance(alloc, mybir_.MemoryLocationSet):
            continue
        name = alloc.memorylocations[0].name
        if alloc.kind == "ExternalInput":
            if name != partition_name:
                in_names.append(name)
        elif alloc.kind == "ExternalOutput":
            out_names.append(name)
            shape = list(alloc.tensor_shape)
            out_avals.append(jax.core.ShapedArray(shape, np.float32))
            zero_outs.append(np.zeros(shape, np.float32))
    n_params = len(in_names)
    n_outs = len(out_names)
    all_names = in_names + out_names
    if partition_name is not None:
        all_names = all_names + [partition_name]

    def _body(*args):
        operands = list(args)
        if partition_name is not None:
            operands.append(bass2jax.partition_id_tensor())
        outs = bass2jax._bass_exec_p.bind(
            *operands,
            out_avals=tuple(out_avals),
            in_names=tuple(all_names),
            out_names=tuple(out_names),
            lowering_input_output_aliases=(),
            sim_require_finite=True,
            sim_require_nnan=True,
            nc=nc,
        )
        return tuple(outs)

    devices = jax.devices()[:NCORES]
    mesh = Mesh(np.asarray(devices), ("core",))
    in_specs = (PartitionSpec("core"),) * (n_params + n_outs)
    out_specs = (PartitionSpec("core"),) * n_outs
    sharded = jax.jit(
        shard_map(_body, mesh=mesh, in_specs=in_specs, out_specs=out_specs, check_rep=False),
        keep_unused=True,
    )

    per_core_vals = {
        "q": [q[NBH * c : NBH * (c + 1)] for c in range(NCORES)],
        "k": [k[NBH * c : NBH * (c + 1)] for c in range(NCORES)],
        "v": [v[NBH * c : NBH * (c + 1)] for c in range(NCORES)],
        "projT": [projT] * NCORES,
    }
    concat_in = [
        np.concatenate(per_core_vals[nm], axis=0) for nm in in_names
    ]
    concat_zeros = [
        np.zeros((NCORES * z.shape[0], *z.shape[1:]), z.dtype) for z in zero_outs
    ]
    sharding = jax.sharding.NamedSharding(mesh, PartitionSpec("core"))
    dev_in = [jax.device_put(a, sharding) for a in concat_in]
    dev_zero = [jax.device_put(a, sharding) for a in concat_zeros]
    r0 = sharded(*dev_in, *dev_zero)
    jax.block_until_ready(r0)
    times = []
    for _ in range(iters):
        t0 = time.perf_counter()
        rr = sharded(*dev_in, *dev_zero)
        jax.block_until_ready(rr)
        times.append(time.perf_counter() - t0)
    out = np.asarray(rr[out_names.index("out")]).reshape(NCORES, NBH, L, D)
    result = out.reshape(B, H, L, D)
    return result, times
